# revision 1
# baseline (speedup 1.0000x reference)
"""BotNet-style multi-head 2D attention with relative position logits, on 8 trn2 cores.

Distribution: data-parallel over batch (B=16 -> 2 per core); all 4 heads +
the rel-pos skew handled on-core.

Per (batch, head) pair the kernel computes, fully on-chip:
    logits = (q*SCALE) @ k^T + skew_w(q @ relw^T) + skew_h(q @ relh^T)
    W      = exp(logits);  W /= rowsum(W)   (softmax without max-subtract:
             logits are O(10) here, exp() is safe in fp32)
    out^T  = V^T @ W^T     (accumulated over key chunks in PSUM)

The rel-pos skew (per-query-row shift) is done with a DRAM round-trip whose
read-back access pattern bakes in the shift, then the per-row [64,128] skewed
tile is added into the logits PSUM via a matmul against a constant 0/1
selector matrix (contraction over the 32 width / 32 height rel positions).
"""

import numpy as np
import ml_dtypes

import concourse.bass as bass
import concourse.mybir as mybir
import concourse.tile as tile
from concourse import bacc
from concourse.ap import AP
from concourse.bass_utils import run_bass_kernel_spmd

FP32 = mybir.dt.float32
BF16 = mybir.dt.bfloat16
AF = mybir.ActivationFunctionType

import os
ABLATE = set(os.environ.get("KERNEL_ABLATE", "").split(","))

NCORES = 8
B_PER_CORE = 2
HEADS = 4
D = 128          # qk and v head dim
C = 512          # input channels
H = W = 32
L = H * W        # 1024 tokens
RC = L // 128    # 8 row chunks of 128 tokens
CC = C // 128    # 4 contraction chunks for the projections
SCALE = D ** (-0.5)
NREL = 2 * W - 1  # 63


def _sel_matrix():
    # sel[k, i*32+j]: k<32 -> (j == k); k>=32 -> (i == k-32)
    # stacked twice (128 partitions) so a skew tile living on partitions
    # 64:128 can use rows 64:128 (matmul needs equal base partitions)
    sel = np.zeros((64, L), np.float32)
    ii, jj = np.divmod(np.arange(L), W)
    for k in range(32):
        sel[k, jj == k] = 1.0
        sel[32 + k, ii == k] = 1.0
    return np.vstack([sel, sel]).astype(ml_dtypes.bfloat16)


def build_bass(iters=1):
    nc = bacc.Bacc()
    fmap = nc.declare_dram_parameter("fmap", [B_PER_CORE, C, L], FP32, isOutput=False)
    wqk = nc.declare_dram_parameter("w_qk", [2 * HEADS * D, C], FP32, isOutput=False)
    wv = nc.declare_dram_parameter("w_v", [HEADS * D, C], FP32, isOutput=False)
    relh = nc.declare_dram_parameter("rel_height", [NREL, D], FP32, isOutput=False)
    relw = nc.declare_dram_parameter("rel_width", [NREL, D], FP32, isOutput=False)
    out = nc.declare_dram_parameter("out", [B_PER_CORE, HEADS * D, L], FP32, isOutput=True)

    sel_const = nc.inline_tensor(_sel_matrix(), name="sel_const")
    ident_const = nc.inline_tensor(np.eye(128, dtype=ml_dtypes.bfloat16), name="ident_const")

    with tile.TileContext(nc) as tc:
        if iters == 1:
            _body(tc, fmap, wqk, wv, relh, relw, out, sel_const, ident_const)
        else:
            with tc.For_i(0, iters, 1):
                _body(tc, fmap, wqk, wv, relh, relw, out, sel_const, ident_const)
    nc.finalize()
    return nc


def _body(tc, fmap, wqk, wv, relh, relw, out, sel_const, ident_const):
    nc = tc.nc
    import contextlib

    ctx = contextlib.ExitStack()
    with ctx:
        persist = ctx.enter_context(tc.tile_pool(name="persist", bufs=1))
        batch_p = ctx.enter_context(tc.tile_pool(name="batch", bufs=2))
        pair_p = ctx.enter_context(tc.tile_pool(name="pair", bufs=2))
        rel_p = ctx.enter_context(tc.tile_pool(name="rel", bufs=4))
        out_p = ctx.enter_context(tc.tile_pool(name="out", bufs=2))
        wt_p = ctx.enter_context(tc.tile_pool(name="wtsb", bufs=3))
        small = ctx.enter_context(tc.tile_pool(name="small", bufs=2))
        dram_p = ctx.enter_context(tc.tile_pool(name="dram", bufs=3, space="DRAM"))

        ps_big = ctx.enter_context(tc.tile_pool(name="ps_big", bufs=2, space="PSUM"))
        ps_wt = ctx.enter_context(tc.tile_pool(name="ps_wt", bufs=2, space="PSUM"))
        ps_av = ctx.enter_context(tc.tile_pool(name="ps_av", bufs=2, space="PSUM"))

        # ---- constants to SBUF ----
        ident = persist.tile([128, 128], BF16, tag="ident")
        nc.sync.dma_start(out=ident, in_=ident_const[:])
        sel = persist.tile([128, L], BF16, tag="sel")
        nc.sync.dma_start(out=sel, in_=sel_const[:])
        identf_const = nc.inline_tensor(np.eye(128, dtype=np.float32))
        identf = persist.tile([128, 128], FP32, tag="identf")
        nc.sync.dma_start(out=identf, in_=identf_const[:])
        # bcast selector: pick8[k, c*128 + p] = (k == c); stationary slice for
        # chunk c replicates rdenT row c across all 128 output partitions
        pick8_np = np.zeros((8, 8 * 128), np.float32)
        for c in range(8):
            pick8_np[c, c * 128:(c + 1) * 128] = 1.0
        pick8_const = nc.inline_tensor(pick8_np.astype(ml_dtypes.bfloat16))
        pick8 = persist.tile([8, 8 * 128], BF16, tag="pick8")
        nc.sync.dma_start(out=pick8, in_=pick8_const[:])

        # ---- weight prep: transpose + cast to bf16 (scale folded into q) ----
        # wqk rows: [0,512) = q (scaled), [512,1024) = k
        # single gpsimd DMA per weight (casts fp32->bf16 in flight):
        # [128, oc*512+c] <- w[oc*128+p, c]
        hwload = "hwload" in ABLATE
        wq_all = persist.tile([128, 8 * C], BF16, tag="wqldb")
        wv_all = persist.tile([128, 4 * C], BF16, tag="wvldb")
        if hwload:
            # HWDGE fp32 loads + engine casts: keeps the (slow) SWDGE
            # descriptor generation off the kernel-start critical path
            stage_p = ctx.enter_context(tc.tile_pool(name="stage", bufs=1))
            wq32 = stage_p.tile([128, 8 * C], FP32, tag="wstage")
            nc.sync.dma_start(
                out=wq32.rearrange("p (a c) -> p a c", a=8),
                in_=wqk[:].rearrange("(a p) c -> p a c", p=128))
            nc.vector.tensor_copy(wq_all, wq32)
            wv32 = stage_p.tile([128, 8 * C], FP32, tag="wstage")
            nc.sync.dma_start(
                out=wv32[:, 0:4 * C].rearrange("p (a c) -> p a c", a=4),
                in_=wv[:].rearrange("(a p) c -> p a c", p=128))
            nc.vector.tensor_copy(wv_all, wv32[:, 0:4 * C])
        else:
            # per-c-chunk loads: the cc=0 weight transposes start after ~1/4
            # of the weight bytes have landed instead of all of them
            for cc in range(CC):
                cs = slice(cc * 128, (cc + 1) * 128)
                nc.gpsimd.dma_start(
                    out=wq_all.rearrange("p (a c) -> p a c", a=8)[:, :, cs],
                    in_=wqk[:].rearrange("(a p) c -> p a c", p=128)[:, :, cs])
                nc.gpsimd.dma_start(
                    out=wv_all.rearrange("p (a c) -> p a c", a=4)[:, :, cs],
                    in_=wv[:].rearrange("(a p) c -> p a c", p=128)[:, :, cs])
        wq_bf = [wq_all[:, oc * C:(oc + 1) * C] for oc in range(8)]
        wv_bf = [wv_all[:, oc * C:(oc + 1) * C] for oc in range(4)]

        wqkT = []   # per cc: [128(c), 1024(o)] bf16, q-half pre-scaled
        for cc in range(CC):
            ps = ps_wt.tile([128, 1024], BF16, tag="ps_wt")
            for oc in range(8):
                nc.tensor.transpose(
                    ps[:, oc * 128:(oc + 1) * 128],
                    wq_bf[oc][:, cc * 128:(cc + 1) * 128],
                    ident,
                )
            t = persist.tile([128, 1024], BF16, tag=f"wqkT{cc}")
            nc.scalar.activation(t[:, 0:512], ps[:, 0:512], AF.Copy, scale=float(SCALE))
            nc.vector.tensor_copy(t[:, 512:1024], ps[:, 512:1024])
            wqkT.append(t)

        wvT = []    # per cc: [128(c), 512(o)] bf16
        for cc in range(CC):
            ps = ps_wt.tile([128, 1024], BF16, tag="ps_wt")
            for oc in range(4):
                nc.tensor.transpose(
                    ps[:, oc * 128:(oc + 1) * 128],
                    wv_bf[oc][:, cc * 128:(cc + 1) * 128],
                    ident,
                )
            t = persist.tile([128, 512], BF16, tag=f"wvT{cc}")
            nc.vector.tensor_copy(t, ps[:, 0:512])
            wvT.append(t)

        # rel tables transposed: [128(d), 63] bf16
        relT_tabs = []
        for name, src in (("relw", relw), ("relh", relh)):
            tbf = small.tile([NREL, D], BF16, tag=f"{name}b")
            nc.gpsimd.dma_start(out=tbf, in_=src[:])
            ps = ps_wt.tile([128, 1024], BF16, tag="ps_wt")
            nc.tensor.transpose(ps[:, 0:NREL], tbf, ident[0:NREL, 0:NREL])
            t = persist.tile([128, NREL], BF16, tag=f"{name}T")
            nc.scalar.activation(t, ps[:, 0:NREL], AF.Copy)
            relT_tabs.append(t)
        relwT, relhT = relT_tabs

        # ---- projections, both batches ----
        qT = {}   # (b, h) -> [128(d), 1024(l)] bf16  (pre-scaled by SCALE)
        kT = {}
        vT = {}   # (b, lc) -> [128(l), 512(h*d)] bf16
        rel_pending = {}  # pair idx -> relwh tile (skew DMA in flight)

        def rel_phase(b, h):
            """q @ rel tables -> skewed+stacked [128, rc, {w,h}, 32] bf16 tiles."""
            ps = ps_big.tile([128, L], FP32, tag="big")
            for rc in range(RC):
                q_ch = qT[(b, h)][:, rc * 128:(rc + 1) * 128]
                nc.tensor.matmul(ps[:, rc * NREL:(rc + 1) * NREL], q_ch, relwT,
                                 start=True, stop=True)
                nc.tensor.matmul(ps[:, 512 + rc * NREL:512 + (rc + 1) * NREL],
                                 q_ch, relhT, start=True, stop=True)
            rel_sb = rel_p.tile([128, 1008], BF16, tag="rel_sb")
            nc.vector.tensor_copy(rel_sb[:, 0:504], ps[:, 0:504])
            nc.scalar.activation(rel_sb[:, 504:1008], ps[:, 512:1016], AF.Copy)
            rd = dram_p.tile([128, 1008], BF16, tag="rel_dram")
            nc.sync.dma_start(out=rd, in_=rel_sb)

            rd_ap = rd[:, :]
            base_t, base_off = rd_ap.tensor, rd_ap.offset
            assert [list(p) for p in rd_ap.ap] == [[1008, 128], [1, 1008]], rd_ap.ap

            relwh = rel_p.tile([128, RC, 2, 32], BF16, tag="relwh")
            with nc.allow_non_contiguous_dma(reason="rel-pos skew gather"):
                # DMA APs are capped at 3 dims, so split the (x_l, y, rc, j)
                # gather by x_l (partition groups of 32).
                for xl in range(4):
                    # width: src elem = p*1008 + rc*63 + (j + 31 - (p%32))
                    src_w = AP(base_t, base_off + xl * 32 * 1008 + 31,
                               [[1008 - 1, 32], [NREL, RC], [1, 32]])
                    nc.sync.dma_start(out=relwh[xl * 32:(xl + 1) * 32, :, 0, :],
                                      in_=src_w)
                    # height: src elem = p*1008 + 504 + rc*63 + (i + 31 - (4*rc + xl))
                    src_h = AP(base_t, base_off + xl * (32 * 1008 - 1) + 504 + 31,
                               [[1008, 32], [NREL - 4, RC], [1, 32]])
                    h_eng = nc.scalar if "hgact" in ABLATE else nc.sync
                    h_eng.dma_start(out=relwh[xl * 32:(xl + 1) * 32, :, 1, :],
                                    in_=src_h)
            return relwh

        fm_bfs = {}

        def proj_qk(b):
            fm_bf = []
            for cc in range(CC):
                fbf = batch_p.tile([128, L], BF16, tag=f"fmbf_{cc}")
                if hwload:
                    f32 = batch_p.tile([128, L], FP32, tag="fm32")
                    nc.sync.dma_start(out=f32,
                                      in_=fmap[b, cc * 128:(cc + 1) * 128, :])
                    nc.vector.tensor_copy(fbf, f32)
                else:
                    nc.gpsimd.dma_start(out=fbf,
                                        in_=fmap[b, cc * 128:(cc + 1) * 128, :])
                fm_bf.append(fbf)
            fm_bfs[b] = fm_bf
            # q/k: out[o_chunk, l] ; o = (q: h*128+d | k: 512 + h*128+d)
            for oc in range(8):
                ps = ps_big.tile([128, L], FP32, tag="big")
                for s in (slice(0, 512), slice(512, 1024)):
                    for cc in range(CC):
                        nc.tensor.matmul(
                            ps[:, s],
                            wqkT[cc][:, oc * 128:(oc + 1) * 128],
                            fm_bf[cc][:, s],
                            start=(cc == 0),
                            stop=(cc == CC - 1),
                        )
                dst = batch_p.tile([128, L], BF16,
                                   tag=f"{'q' if oc < 4 else 'k'}T{oc % 4}")
                if oc >= 4 and "kdve" in ABLATE:
                    nc.vector.tensor_copy(dst, ps)
                else:
                    nc.scalar.activation(dst, ps, AF.Copy)
                if oc < 4:
                    qT[(b, oc)] = dst
                else:
                    kT[(b, oc - 4)] = dst

        def proj_v_blocks(b):
            # v^T: out[l_chunk, h*d]; generator so batch 1's v projections can
            # interleave with pair 0's (ACT-bound) logits+softmax phase
            fm_bf = fm_bfs[b]
            for lc in range(RC):
                ps = ps_big.tile([128, L], FP32, tag="big")
                for cc in range(CC):
                    nc.tensor.matmul(
                        ps[:, 0:512],
                        fm_bf[cc][:, lc * 128:(lc + 1) * 128],
                        wvT[cc],
                        start=(cc == 0),
                        stop=(cc == CC - 1),
                    )
                dst = batch_p.tile([128, 512], BF16, tag=f"vT{lc}")
                nc.vector.tensor_copy(dst, ps[:, 0:512])
                vT[(b, lc)] = dst
                yield

        # ---- attention pairs ----
        pairs = [(b, h) for b in range(B_PER_CORE) for h in range(HEADS)]

        def main_blocks(b, h, relwh, result):
            # generator: yields after each row-chunk block so the driver can
            # interleave this (ACT-bound) phase with the previous pair's
            # (PE-bound) av phase
            # skewed rel tiles [64,128] via PE transpose + DVE copy, staggered
            # through the qk/sel matmul stream so the PE never queues up
            # behind the (slower) DVE copies
            relTs = [None] * RC

            def rel_tile(rc2):
                # two row-chunks per PE transpose + one DVE copy: the [128,128]
                # result holds rc2's [64,128] skew tile on partitions 0:64 and
                # rc2+1's on 64:128
                psr = ps_wt.tile([128, 128], BF16, tag="ps_wt")
                nc.tensor.transpose(
                    psr,
                    relwh[:, rc2:rc2 + 2, :, :].rearrange("p a b c -> p (a b c)"),
                    ident)
                relT2 = small.tile([128, 128], BF16, tag=f"relT{rc2}")
                nc.vector.tensor_copy(relT2, psr)
                relTs[rc2] = relT2[0:64, :]
                relTs[rc2 + 1] = relT2[64:128, :]

            rel_tile(0)
            Wt = []
            den_all = small.tile([128, RC], FP32, tag="den_all")
            for rc in range(RC):
                ps_l = ps_big.tile([128, L], FP32, tag="big")
                q_ch = qT[(b, h)][:, rc * 128:(rc + 1) * 128]
                for s in (slice(0, 512), slice(512, 1024)):
                    nc.tensor.matmul(ps_l[:, s], q_ch, kT[(b, h)][:, s],
                                     start=True, stop=False)
                selh = sel[0:64, :] if rc % 2 == 0 else sel[64:128, :]
                for s in (slice(0, 512), slice(512, 1024)):
                    nc.tensor.matmul(ps_l[:, s], relTs[rc], selh[:, s],
                                     start=False, stop=True)
                if rc % 2 == 0 and rc + 2 < RC:
                    rel_tile(rc + 2)
                w_sb = pair_p.tile([128, L], BF16, tag=f"W{rc}")
                nc.scalar.activation(w_sb, ps_l, AF.Exp,
                                     accum_out=den_all[:, rc:rc + 1])
                if "defernorm" not in ABLATE:
                    rden1 = small.tile([128, 1], FP32, tag=f"rden{rc}")
                    nc.vector.reciprocal(rden1, den_all[:, rc:rc + 1])
                    nc.vector.tensor_scalar_mul(w_sb, w_sb, rden1)
                Wt.append(w_sb)
                yield
            result.extend((Wt, den_all))

        def av_blocks(b, h, Wt, den_all, o_sb):
            # two single-bank accumulators: the next head's first AV matmul
            # reuses bank 0 while this head's second half is still being
            # read out by the final normalize multiply
            ps_o0 = ps_av.tile([128, 512], FP32, tag="ps_av")
            ps_o1 = ps_av.tile([128, 512], FP32, tag="ps_av")
            ps_o = {0: ps_o0, 1: ps_o1}
            rden_bc = None
            if "defernorm" in ABLATE:
                # reciprocal denominators -> [rc, x-chunk] layout -> broadcast
                # to [128, 1024] while the AV matmuls run; consumed by the
                # final column-scaled PSUM->SBUF multiply
                rden = small.tile([128, RC], FP32, tag="rden")
                nc.vector.reciprocal(rden, den_all)
                psr8 = ps_wt.tile([8, 128], FP32, tag="ps_wt")
                nc.tensor.transpose(psr8, rden, identf)
                rdenT = small.tile([8, 128], BF16, tag="rdenT")
                nc.vector.tensor_copy(rdenT, psr8)
                rden_bc = wt_p.tile([128, L], BF16, tag="rdbc")
                for half in range(2):
                    psb = ps_wt.tile([128, 512], FP32, tag="ps_wt")
                    for c4 in range(4):
                        c = half * 4 + c4
                        nc.tensor.matmul(psb[:, c4 * 128:(c4 + 1) * 128],
                                         pick8[:, c * 128:(c + 1) * 128], rdenT,
                                         start=True, stop=True)
                    nc.vector.tensor_copy(rden_bc[:, half * 512:(half + 1) * 512], psb)

            def transpose_group(cc):
                ps_w = ps_wt.tile([128, 1024], BF16, tag="ps_wt")
                for rc in range(RC):
                    nc.tensor.transpose(
                        ps_w[:, rc * 128:(rc + 1) * 128],
                        Wt[rc][:, cc * 128:(cc + 1) * 128],
                        ident,
                    )
                wt_sb = wt_p.tile([128, 1024], BF16, tag="wt")
                nc.vector.tensor_copy(wt_sb, ps_w)
                return wt_sb

            # two transpose groups ahead of the av matmuls so the PE never
            # waits on the PSUM->SBUF copy of the group it is about to consume
            wts = [transpose_group(0), transpose_group(1)]
            for cc in range(RC):
                if cc + 2 < RC:
                    wts.append(transpose_group(cc + 2))
                wt_cur = wts[cc]
                v_ch = vT[(b, cc)][:, h * 128:(h + 1) * 128]
                for half in range(2):
                    nc.tensor.matmul(ps_o[half],
                                     v_ch, wt_cur[:, half * 512:(half + 1) * 512],
                                     start=(cc == 0), stop=(cc == RC - 1))
                yield
            for half in range(2):
                s = slice(half * 512, (half + 1) * 512)
                dst = o_sb[:, h * L + half * 512:h * L + half * 512 + 512]
                if "defernorm" not in ABLATE:
                    if half:
                        nc.scalar.activation(dst, ps_o[half], AF.Copy)
                    else:
                        nc.vector.tensor_copy(dst, ps_o[half])
                else:
                    nc.vector.scalar_tensor_tensor(
                        out=dst, in0=ps_o[half], scalar=1.0, in1=rden_bc[:, s],
                        op0=mybir.AluOpType.mult, op1=mybir.AluOpType.mult)
            # per-head store: out[b, h*128+d, l] <- o_sb[d, h*L + l]; the
            # transfer overlaps the remaining heads' compute and stays off
            # the ACT queue (whose DGE-config time would delay the exps)
            out_ap = AP(out[b].tensor, out[b].offset + h * 128 * L,
                        [[L, 128], [1, L]])
            nc.sync.dma_start(out=out_ap, in_=o_sb[:, h * L:(h + 1) * L])
            yield

        # ---- drive: projections, then software-pipelined pair loop ----
        # av(i-1) (PE-bound: transposes + AV matmuls, ACT mostly idle) is
        # interleaved block-by-block with main(i) (ACT-bound: exp per row
        # chunk, PE ~60% idle), so each engine sees a steady mix. rel skew
        # round trips stay 2 pairs ahead. Pair 0's main interleaves with
        # batch 1's v projections for the same reason.
        proj_qk(0)
        rel_pending[0] = rel_phase(0, 0)
        rel_pending[1] = rel_phase(0, 1)
        for _ in proj_v_blocks(0):
            pass
        proj_qk(1)
        for _ in proj_v_blocks(1):
            pass

        o_sb_map = {}
        prev_av = None
        for i, (b, h) in enumerate(pairs):
            if h == 0:
                o_sb_t = out_p.tile([128, HEADS * L], FP32, tag="o_sb")
                o_sb_map[b] = o_sb_t
            res = []
            gm = main_blocks(b, h, rel_pending.pop(i), res)
            if prev_av is None:
                for _ in gm:
                    pass
            else:
                for _ in range(RC):
                    next(prev_av)
                    next(gm)
                for _ in prev_av:
                    pass
                for _ in gm:
                    pass
            if i + 2 < len(pairs):
                rel_pending[i + 2] = rel_phase(*pairs[i + 2])
            Wt, den_all = res
            prev_av = av_blocks(b, h, Wt, den_all, o_sb_map[b])
        for _ in prev_av:
            pass


_NC_CACHE = None


def get_nc():
    global _NC_CACHE
    if _NC_CACHE is None:
        _NC_CACHE = build_bass()
    return _NC_CACHE


def kernel(featuremap, w_qk, w_v, rel_height, rel_width):
    B, C_, H_, W_ = featuremap.shape
    nc = get_nc()
    fm = np.ascontiguousarray(featuremap, np.float32).reshape(B, C_, H_ * W_)
    common = {
        "w_qk": np.ascontiguousarray(w_qk, np.float32),
        "w_v": np.ascontiguousarray(w_v, np.float32),
        "rel_height": np.ascontiguousarray(rel_height, np.float32),
        "rel_width": np.ascontiguousarray(rel_width, np.float32),
    }
    in_maps = [
        {"fmap": fm[i * B_PER_CORE:(i + 1) * B_PER_CORE], **common}
        for i in range(NCORES)
    ]
    res = run_bass_kernel_spmd(nc, in_maps, list(range(NCORES))).results
    outs = [res[i]["out"].reshape(B_PER_CORE, HEADS * D, H_, W_) for i in range(NCORES)]
    return np.concatenate(outs, axis=0).astype(np.float32)

